# revision 31
# baseline (speedup 1.0000x reference)
"""AddContextFrames distributed Trainium2 kernel.

Reference op: out[0, w*80+f, t] = signal[0, f, t + w - 9] (zero outside),
w in 0..19 — i.e. the output stacks 19 time-shifted copies of the input.
Pure data movement; memory-bound (199 MB output from a 10.5 MB input).

Distribution: shard the time axis across 8 NeuronCores.  Each core's input
shard (80, 4096+18) is built host-side from the zero-padded full signal, so
the halo is included and no inter-core communication is needed.

Per-core kernel (4 DMAs total):
  SBUF layout: each feature row is split into 2 time blocks of 2048
  (sub-row s = 2f + b holds x[f, b*2048 : b*2048 + 2066], incl. 18-elem
  halo).  Region 0: sub-rows 0..127 on partition s (slot 0).  Region 1:
  sub-rows 128..159 on partitions 4j (slot 1) — 2 per SBUF AXI port, so
  all 16 ports / SDMA engines carry equal load.
  - 2 loads (region 0: 1.05 MB, region 1: 0.26 MB)
  - 3 stores (region 1; region 0 in two window-halves), each covering
    many windows in a single 3D access pattern
    [[partition, N], [1, n_windows], [1, 2048]] whose element order
    matches the fully contiguous DRAM output.  8 KB descriptors, minimal
    per-DMA overhead, uniform descriptor dealing across all 16 SDMA
    engines (only full-128 / stride-4-base-0-32 partition shapes deal
    uniformly; anything else concentrates on low-numbered engines).
Measured on TRN2: ~74-88 us per NEFF execution (min 73.5 us), against a
~70 us HBM roofline (23.75 MB of output writes per core at ~360 GB/s).
The ~74/~87 bimodality is SDMA engine 15 running ~1.19x slower than its
peers in about half of runs — not addressable via work distribution given
the descriptor-dealing constraints.  Interleaved A/B tests showed store
granularity (3 vs 6 vs 11 stores), load splitting, and ACT-ring load
issue are all within noise of this version.
"""

import numpy as np

import concourse.bass as bass
import concourse.mybir as mybir
from concourse.bass_utils import run_bass_kernel_spmd

N_CORES = 8
N_CONTEXT = 9
WINDOW = 2 * N_CONTEXT + 1  # 19
FEATS = 80
STEPS = 32768
SHARD = STEPS // N_CORES    # 4096
HALO = 2 * N_CONTEXT        # 18
IN_W = SHARD + HALO         # 4114
OUT_CH = WINDOW * FEATS     # 1520

TB = 2048            # time block per sub-row
SUBW = TB + HALO     # 2066 elements stored per sub-row
PITCH = 2072         # sub-row pitch in elements (32-byte aligned)

_nc_cache = None


def build_nc() -> bass.Bass:
    from concourse.ap import AP

    nc = bass.Bass()
    x = nc.declare_dram_parameter(
        "signal", [FEATS, IN_W], mybir.dt.float32, isOutput=False
    )
    out = nc.declare_dram_parameter(
        "out", [OUT_CH, SHARD], mybir.dt.float32, isOutput=True
    )
    with (
        nc.sbuf_tensor([128, 2, PITCH], mybir.dt.float32) as tile,
        nc.semaphore("ld0") as ld0,
        nc.semaphore("ld1") as ld1,
        nc.semaphore("ss") as ss,
        nc.Block() as block,
    ):
        th = tile.tensor if hasattr(tile, "tensor") else tile
        FS = FEATS * SHARD
        PP = 2 * PITCH  # flat elements per partition

        # region-0 load on the scalar (ACT) HWDGE ring so it streams in
        # parallel with the region-1 load instead of queueing behind it —
        # releases the big store ~2 us earlier (measured ~3-5 us end-to-end
        # in phase-matched A/B runs).
        @block.scalar
        def _(scalar):
            # region-0 load: sub-row s = (f, b) = (s//2, s%2) -> partition s
            scalar.dma_start(
                out=tile[:, 0, 0:SUBW],
                in_=AP(x, 0, [[IN_W, 64], [TB, 2], [1, SUBW]]),
            ).then_inc(ld0, 16)

        @block.sync
        def _(sync):
            # region-1 load: sub-row 128+j = (f, b) = (64 + j//2, j%2)
            # -> partition 4j slot 1; smallest load, gates the first store.
            sync.dma_start(
                out=tile[0:128:4, 1, 0:SUBW],
                in_=AP(x, 64 * IN_W, [[IN_W, 16], [TB, 2], [1, SUBW]]),
            ).then_inc(ld1, 16)
            # stores: DRAM element index = w*FS + s*TB + t equals the SBUF
            # element order (partition, window, time) of a 3D AP — one DMA
            # per region covers all 19 windows.
            sync.wait_ge(ld1, 16)
            sync.dma_start(
                out=AP(out, 128 * TB, [[TB, 32], [FS, WINDOW], [1, TB]]),
                in_=AP(th, PITCH, [[4 * PP, 32], [1, WINDOW], [1, TB]]),
            ).then_inc(ss, 16)
            sync.wait_ge(ld0, 16)
            sync.dma_start(
                out=AP(out, 0, [[TB, 128], [FS, 10], [1, TB]]),
                in_=AP(th, 0, [[PP, 128], [1, 10], [1, TB]]),
            ).then_inc(ss, 16)
            sync.dma_start(
                out=AP(out, 10 * FS, [[TB, 128], [FS, 9], [1, TB]]),
                in_=AP(th, 10, [[PP, 128], [1, 9], [1, TB]]),
            ).then_inc(ss, 16)
            sync.wait_ge(ss, 48)

    return nc


def _install_ntff_hook():
    """The image lacks antenv.axon_hooks; synthesize it so trace=True works."""
    import sys, types

    if "antenv.axon_hooks" in sys.modules:
        return
    try:
        from trn_agent_boot.trn_boot import _ntff_profile_via_ctypes

        mod = types.ModuleType("antenv.axon_hooks")
        _state = {"hook": _ntff_profile_via_ctypes("/opt/axon/libaxon_pjrt.so")}
        mod.get_axon_ntff_profile_hook = lambda: _state["hook"]
        mod.set_axon_ntff_profile_hook = lambda h: _state.__setitem__("hook", h)
        sys.modules["antenv.axon_hooks"] = mod
        import antenv

        antenv.axon_hooks = mod
    except Exception:
        pass


def run(signal: np.ndarray, trace: bool = False):
    """signal: (1, 80, 32768) f32 -> ((1, 1520, 32768) f32, BassKernelResults)"""
    global _nc_cache
    if trace:
        _install_ntff_hook()
    signal = np.asarray(signal, dtype=np.float32)
    xp = np.zeros((FEATS, STEPS + HALO), np.float32)
    xp[:, N_CONTEXT : N_CONTEXT + STEPS] = signal[0]
    in_maps = [
        {"signal": np.ascontiguousarray(xp[:, i * SHARD : i * SHARD + IN_W])}
        for i in range(N_CORES)
    ]
    if _nc_cache is None:
        _nc_cache = build_nc()
    res = run_bass_kernel_spmd(
        _nc_cache, in_maps, core_ids=list(range(N_CORES)), trace=trace
    )
    out = np.empty((1, OUT_CH, STEPS), np.float32)
    for i in range(N_CORES):
        out[0, :, i * SHARD : (i + 1) * SHARD] = np.asarray(res.results[i]["out"])
    return out, res


def kernel(signal: np.ndarray) -> np.ndarray:
    out, _ = run(signal, trace=False)
    return out
